# revision 37
# baseline (speedup 1.0000x reference)
"""Trainium2 Bass kernel for BiLinearInteractionLayer.

Computes, for every field pair p=(i,j), i<j, of F=32 fields:
    y[b, p, :] = (x[b, i, :] @ W[p].T) * x[b, j, :]
x: [4096, 32, 64] f32, W: [496, 64, 64] f32 -> y: [4096, 496, 64] f32.

Sharding: data-parallel over the batch dim across 8 NeuronCores (512
rows each); the weight stack is replicated.

Per-core algorithm (batch tile of 128 rows at a time):
  - Host pre-transposes layouts and downcasts to bf16 (free): the
    contraction dim d lands on SBUF partitions with contiguous DMAs.
    bf16 inputs + bf16 output (upconverted to f32 on the host) halve
    all HBM traffic; measured error ~7e-3 of output scale vs the 2e-2
    harness gate.
  - For each first-field i, the pairs (i, i+1..31) are contiguous both
    in the pair axis and in the transposed weight columns: one
    stationary xT_i [64d, 128b] serves matmuls streaming W^T columns
    (N<=512 per PSUM bank) into [128, <=1024] PSUM groups. Even fields
    use PE rows 0-63, odd fields rows 64-127.
  - PSUM: 2 tags (parity) x 2 banks x bufs=2 fills all 8 banks and
    breaks the matmul(g+1)-waits-mul(g) ping-pong chain that otherwise
    dominates the span.
  - The elementwise xj multiply is split between DVE (tensor_mul
    straight from PSUM, 1.042 ns/col) and gpsimd (ACT first drains
    PSUM->SBUF since the Q7 cores can't read PSUM; ~1.98 ns/col),
    greedily balanced by measured ns/col. An ACT+DVE-2x_1p bf16 route
    exists (routes3) but loses to SBUF port contention in practice.
  - Outputs stage into one SBUF buffer per parity per tile (parity-major
    HBM layout, unshuffled on the host) and drain in ~quarter-parity
    DMA chunks: 124 -> 40 output DMAs, keeping the SP sequencer (~600ns
    per dma_start) off the critical path. Input loads issue from the
    scalar queue so they are never FIFO-blocked behind output stores.
"""

import itertools

import numpy as np

import concourse.bass as bass
import concourse.mybir as mybir
import concourse.tile as _tile
from concourse.bass_utils import run_bass_kernel_spmd
from concourse.tile import TileContext
from concourse.tile_scheduler import N_PROCS
from concourse.vector_clock import ScopedClock, VectorClock

# --------------------------------------------------------------------------
# Tail-drain patch: the staged walrus rejects >1 sync-wait command on a
# TPB_CTRL (Drain) instruction, but the stock Tile tail-drain attaches one
# wait per outstanding sem lane to a single Drain. Replace it with a ladder
# of single-wait SP nops (one per proc lane) followed by a wait-less drain.
# --------------------------------------------------------------------------


def _split_drain_and_barrier(self, tick_clock, wait_clock):
    nc = self.nc
    g = tick_clock.global_clock
    for p in range(N_PROCS):
        tick = g.peek_next(p) - 1
        if tick <= 0:
            continue
        pc = VectorClock()
        pc.require_at_least(p, tick)
        w = nc.sync.nop(nofuse=True)
        wait_clock.add_sem_waits(w.ins, ScopedClock({None: pc}))
    nc.sync.drain()
    nc.all_engine_barrier()
    assert self.sems is not None
    popped = nc._tile_sem_poison_stack.pop()
    assert popped is self._sem_poison
    nc.clear_and_free_semaphores(list(self.sems.allocated().values()))
    nc.all_engine_barrier()


_tile.TileContext._drain_and_barrier = _split_drain_and_barrier

_wsplit_counter = [0]


def _legalize_single_wait(nc):
    """Hoist extra sem waits onto preceding same-engine NoOps.

    This walrus build encodes at most ONE sync-wait command per TPB
    instruction; Tile's sem-assignment pass freely attaches several.
    Splitting extras onto immediately-preceding NoOps on the same engine
    preserves program order (engines issue in order), hence semantics."""
    import bass_rust

    for fn in nc.m.functions:
        for blk in fn.blocks:
            insts = list(blk.instructions)
            if not any(
                ins.sync_info is not None and len(ins.sync_info.on_wait) > 1
                for ins in insts
            ):
                continue
            out = []
            for ins in insts:
                si = ins.sync_info
                waits = list(si.on_wait) if si is not None else []
                if len(waits) > 1:
                    for w in waits[:-1]:
                        _wsplit_counter[0] += 1
                        nop = mybir.InstNoOp(
                            name=f"I-wsplit-{_wsplit_counter[0]}", ins=[], outs=[]
                        )
                        nop.engine = ins.engine
                        nop.sync_info = bass_rust.SyncInfo(
                            on_wait=[w], on_update=[]
                        )
                        out.append(nop)
                    si.on_wait = [waits[-1]]
                out.append(ins)
            blk.instructions = out


# --------------------------------------------------------------------------
# Problem constants (hardcoded per contract: kernel.py is self-contained).
# --------------------------------------------------------------------------
B, F, D = 4096, 32, 64
NCORES = 8
BL = B // NCORES          # 512 batch rows per core
PT = 128                  # batch tile = SBUF partition count
TILES = BL // PT          # 4 tiles per core
NPAIR = F * (F - 1) // 2  # 496
# pair index of (i, i+1) within itertools.combinations(range(F), 2) order
IDX0 = [0] * F
for _i in range(1, F):
    IDX0[_i] = IDX0[_i - 1] + (F - _i)
# per-parity column offset of field i's run inside its wt half
POFF = [0] * F
for _i in range(2, F):
    POFF[_i] = POFF[_i - 2] + (F - 1 - (_i - 2)) * D
WT_COLS = max(POFF[30] + 1 * D, POFF[31])  # even half is the larger: 16384
WT_COLS = max(WT_COLS, 16384)

F32 = mybir.dt.float32

_nc_cache = {}


PAR_COLS = (16384, 15360)   # total columns per parity half (even-i, odd-i)
PAR_BASE = (0, 16384)       # column base of each parity in the merged layout


def _build_bass(mm_dt=F32, psum_cols=1984, psum_bufs=1, io_bufs=2, out_bufs=3,
                derive_x=False, x_dt=None, gp_frac=0.0, in_eng="sync",
                gp_self_dma=False, out_dt=None, merged_out=False,
                routes3=False):
    if x_dt is None:
        x_dt = F32
    if out_dt is None:
        out_dt = F32
    nc = bass.Bass(trn_type="TRN2")
    if derive_x:
        # x (natural layout) is rebuilt on-chip from xt via PE transposes,
        # saving its 4 MB HBM read; 2 PSUM banks go to transpose staging
        psum_cols = 1536
        x_d = None
        id_d = nc.dram_tensor("ident", [PT, PT], mm_dt, kind="ExternalInput")
    else:
        x_d = nc.dram_tensor("x", [BL, F * D], x_dt, kind="ExternalInput")
    xt_d = nc.dram_tensor("xt", [PT, TILES * (F // 2) * PT], mm_dt,
                          kind="ExternalInput")
    wt_d = nc.dram_tensor("wt", [PT, WT_COLS], mm_dt, kind="ExternalInput")
    y_d = nc.dram_tensor("y", [BL, NPAIR * D], out_dt, kind="ExternalOutput")

    CB = (F // 2) * PT  # 2048 xt cols per batch tile

    with TileContext(nc) as tc:
        # Elementwise-multiply routing. Three pipelines (ns/col measured
        # on HW via microbench, + per-instruction fixed cost):
        #   A: DVE mul straight from PSUM           (DVE 1.042)
        #   B: ACT copy PSUM->bf16 SBUF, DVE mul    (ACT 1.056, DVE 0.521:
        #      all-bf16 SBUF operands engage DVE 2x_1p)
        #   C: ACT copy PSUM->bf16 SBUF, gpsimd mul (ACT 1.056, GP ~1.98)
        # Greedy makespan balance across the three engine clocks.
        eng_t = {"dve": 0.0, "act": 0.0, "gp": 0.0}

        def _route(gcols, allow_b=routes3):
            if gp_frac <= 0.0:
                return "A"
            cost = {
                "A": {"dve": gcols * 1.042 + 150},
                "B": {"act": gcols * 1.056 + 150, "dve": gcols * 0.521 + 150},
                "C": {"act": gcols * 1.056 + 150, "gp": gcols * gp_frac + 450},
            }
            if not allow_b:
                del cost["B"]
            best, best_mk = None, None
            for r, c in cost.items():
                mk = max(
                    eng_t[e] + c.get(e, 0.0) for e in eng_t
                )
                if best_mk is None or mk < best_mk:
                    best, best_mk = r, mk
            for e, v in cost[best].items():
                eng_t[e] += v
            return best

        with (
            tc.tile_pool(name="wtp", bufs=1) as wtp,
            tc.tile_pool(name="iop", bufs=io_bufs) as iop,
            tc.tile_pool(name="outp", bufs=out_bufs) as outp,
            tc.tile_pool(name="projp", bufs=2) as projp,
            tc.tile_pool(name="pp", bufs=psum_bufs, space="PSUM") as pp,
        ):
            # input loads go out on their own queue (in_eng) so they are
            # never FIFO-blocked behind output stores on the sync queue
            ieng = getattr(nc, in_eng)
            if derive_x:
                ident = wtp.tile([PT, PT], mm_dt, tag="id")
                ieng.dma_start(out=ident, in_=id_d[:])
            # tile-0 inputs are issued ahead of the weight chunks so the
            # first matmuls aren't gated on the whole weight stream
            x0_s = None
            xt0_s = iop.tile([PT, CB], mm_dt, tag="xt", name="xt_0")
            ieng.dma_start(out=xt0_s, in_=xt_d[:, 0:CB])
            wt_s = wtp.tile([PT, WT_COLS], mm_dt)
            # chunked weight load: matmuls for early fields only depend on
            # their own column range (Tile subtile deps), so compute starts
            # once the first slivers land instead of after all 4 MB. The
            # first chunks are small (512 cols = one matmul's worth) and
            # x0 is interleaved right after the first one, so the chain
            # "first matmul -> first mul" unblocks ~4 us earlier.
            wt_chunks = [512, 512, 1024] + [2048] * ((WT_COLS - 2048) // 2048)
            w0 = 0
            for ci, wch in enumerate(wt_chunks):
                ieng.dma_start(
                    out=wt_s[:, w0 : w0 + wch], in_=wt_d[:, w0 : w0 + wch]
                )
                w0 += wch
                if ci == 0 and not derive_x:
                    x0_s = iop.tile([PT, F * D], x_dt, tag="x", name="x_0")
                    ieng.dma_start(out=x0_s, in_=x_d[0:PT, :])
            if derive_x:
                pass
            elif x0_s is None:
                x0_s = iop.tile([PT, F * D], x_dt, tag="x", name="x_0")
                ieng.dma_start(out=x0_s, in_=x_d[0:PT, :])
            for t in range(TILES):
                if t == 0:
                    xt_s = xt0_s
                    x_s = x0_s
                    if derive_x:
                        x_s = iop.tile([PT, F * D], F32, tag="x", name="x_0")
                else:
                    x_s = iop.tile(
                        [PT, F * D], x_dt if not derive_x else F32, tag="x",
                        name=f"x_{t}",
                    )
                    if not derive_x:
                        ieng.dma_start(
                            out=x_s, in_=x_d[t * PT : (t + 1) * PT, :]
                        )
                    xt_s = iop.tile([PT, CB], mm_dt, tag="xt", name=f"xt_{t}")
                    ieng.dma_start(
                        out=xt_s, in_=xt_d[:, t * CB : (t + 1) * CB]
                    )
                if derive_x:
                    # x[b, (2m+par)*64+d] = xt[par*64+d, m*128+b]: one PE
                    # transpose per 128-column chunk, ACT copies PSUM->SBUF
                    for m in range(F // 2):
                        tp = pp.tile(
                            [PT, PT], mm_dt, tag="tp", bufs=2,
                            name=f"tp_{t}_{m}",
                        )
                        nc.tensor.transpose(
                            tp, xt_s[:, m * PT : (m + 1) * PT], ident
                        )
                        nc.scalar.copy(
                            out=x_s[:, m * PT : (m + 1) * PT], in_=tp
                        )
                if merged_out:
                    # one SBUF staging buffer per parity for the whole
                    # tile; groups land at their parity-local POFF offset
                    # and only ~2 big DMAs per parity drain it (the DMA
                    # count is decoupled from the PSUM group size)
                    om = {
                        par: outp.tile(
                            [PT, PAR_COLS[par]], out_dt, tag=f"o{par}",
                            name=f"o_{t}_{par}",
                        )
                        for par in (0, 1)
                    }
                    flushed = {0: 0, 1: 0}

                    def _flush(par, upto, t=t, om=om, flushed=flushed):
                        a = flushed[par]
                        if upto <= a:
                            return
                        nc.sync.dma_start(
                            out=y_d[
                                t * PT : (t + 1) * PT,
                                PAR_BASE[par] + a : PAR_BASE[par] + upto,
                            ],
                            in_=om[par][:, a:upto],
                        )
                        flushed[par] = upto
                for m in range(F // 2):
                    # split each live parity's field run into PSUM-group
                    # work items of <= psum_cols columns, then alternate
                    # parities so the two PE row groups interleave
                    per_par = []
                    for par in (0, 1):
                        i = 2 * m + par
                        if i > F - 2:
                            continue
                        ncol = (F - 1 - i) * D
                        groups = []
                        for g0 in range(0, ncol, psum_cols):
                            gcols = min(psum_cols, ncol - g0)
                            groups.append((par, i, g0, gcols))
                        per_par.append(groups)
                    order = [
                        g
                        for pair in itertools.zip_longest(*per_par)
                        for g in pair
                        if g is not None
                    ]
                    for par, i, g0, gcols in order:
                        lhsT = xt_s[par * D : (par + 1) * D,
                                    m * PT : (m + 1) * PT]
                        off = POFF[i] + g0
                        ps = pp.tile(
                            [PT, psum_cols], F32, tag=f"ps{par}",
                            name=f"ps_{t}_{i}_{g0}",
                        )
                        if merged_out:
                            out_s = om[par][:, off : off + gcols]
                        else:
                            ot = outp.tile(
                                [PT, psum_cols], out_dt, tag=f"o{par}",
                                name=f"o_{t}_{i}_{g0}",
                            )
                            out_s = ot[:, :gcols]
                        for k0 in range(0, gcols, 512):
                            kn = min(512, gcols - k0)
                            nc.tensor.matmul(
                                ps[:, k0 : k0 + kn],
                                lhsT,
                                wt_s[par * D : (par + 1) * D,
                                     off + k0 : off + k0 + kn],
                                start=True,
                                stop=True,
                            )
                        c0 = IDX0[i] * D + g0
                        xj = x_s[:, (i + 1) * D + g0 : (i + 1) * D + g0 + gcols]
                        route = _route(gcols)
                        if route == "A":
                            nc.vector.tensor_mul(
                                out=out_s, in0=ps[:, :gcols], in1=xj
                            )
                        else:
                            proj_s = projp.tile(
                                [PT, psum_cols],
                                out_dt if routes3 else F32,
                                tag=f"pr{par}",
                                name=f"pr_{t}_{i}_{g0}",
                            )
                            nc.scalar.copy(
                                out=proj_s[:, :gcols], in_=ps[:, :gcols]
                            )
                            meng = nc.vector if route == "B" else nc.gpsimd
                            meng.tensor_mul(
                                out=out_s, in0=proj_s[:, :gcols], in1=xj
                            )
                        if merged_out:
                            end = off + gcols
                            # drain the staging buffer in ~quarter chunks so
                            # the output stream starts early in each tile;
                            # eighths on the last tile so the final DMA
                            # after the last mul is small (shorter tail)
                            step = PAR_COLS[par] // 8
                            nxt = (flushed[par] // step + 1) * step
                            if end >= nxt and end < PAR_COLS[par]:
                                _flush(par, end)
                        else:
                            nc.sync.dma_start(
                                out=y_d[
                                    t * PT : (t + 1) * PT, c0 : c0 + gcols
                                ],
                                in_=out_s,
                            )
                if merged_out:
                    for par in (0, 1):
                        _flush(par, PAR_COLS[par])
    _legalize_single_wait(nc)
    return nc


def _get_nc(mm_dt, psum_cols, psum_bufs, io_bufs=2, out_bufs=3, derive_x=False,
            x_dt=None, gp_frac=0.0, in_eng="sync", gp_self_dma=False,
            out_dt=None, merged_out=False, routes3=False):
    key = (str(mm_dt), psum_cols, psum_bufs, io_bufs, out_bufs, derive_x,
           str(x_dt), gp_frac, in_eng, gp_self_dma, str(out_dt), merged_out,
           routes3)
    if key not in _nc_cache:
        _nc_cache[key] = _build_bass(
            mm_dt, psum_cols, psum_bufs, io_bufs, out_bufs, derive_x, x_dt,
            gp_frac, in_eng, gp_self_dma, out_dt, merged_out, routes3,
        )
    return _nc_cache[key]


def _np_dt(dt):
    return mybir.dt.np(dt)


def _prep_inputs(x, W, derive_x=False, mm_dt=F32, x_dt=None):
    if x_dt is None:
        x_dt = F32
    mm_np = _np_dt(mm_dt)
    x_np = _np_dt(x_dt)
    x = np.ascontiguousarray(np.asarray(x, dtype=np.float32))
    W = np.ascontiguousarray(np.asarray(W, dtype=np.float32))
    # wt2[par*64+d, POFF[i] + (j-i-1)*64 + o] = W[(i,j), o, d]
    wt2 = np.zeros((PT, WT_COLS), dtype=np.float32)
    for i in range(F - 1):
        par = i % 2
        npair = F - 1 - i
        blk = W[IDX0[i] : IDX0[i] + npair]           # [npair, D, D]
        blk = blk.transpose(2, 0, 1).reshape(D, npair * D)
        wt2[par * D : (par + 1) * D, POFF[i] : POFF[i] + npair * D] = blk
    wt2 = np.ascontiguousarray(wt2.astype(mm_np))
    in_maps = []
    for c in range(NCORES):
        xl = x[c * BL : (c + 1) * BL]                      # [512, 32, 64]
        x_in = np.ascontiguousarray(xl.reshape(BL, F * D).astype(x_np))
        # xt2[par*64+d, t*2048 + m*128 + b] = xl[t*128+b, 2m+par, d]
        xt2 = np.ascontiguousarray(
            xl.reshape(TILES, PT, F // 2, 2, D).transpose(3, 4, 0, 2, 1)
        ).reshape(PT, TILES * (F // 2) * PT).astype(mm_np)
        xt2 = np.ascontiguousarray(xt2)
        m = {"xt": xt2, "wt": wt2}
        if derive_x:
            m["ident"] = np.eye(PT, dtype=mm_np)
        else:
            m["x"] = x_in
        in_maps.append(m)
    return in_maps


def _unmerge(yd):
    """Undo the parity-major device layout -> reference pair order."""
    out = np.empty((BL, NPAIR * D), dtype=np.float32)
    for i in range(F - 1):
        run = (F - 1 - i) * D
        src = PAR_BASE[i % 2] + POFF[i]
        out[:, IDX0[i] * D : IDX0[i] * D + run] = yd[:, src : src + run]
    return out


def _run(x, W, trace=False, mm_dt=None, psum_cols=1984, psum_bufs=1, io_bufs=2,
         out_bufs=3, derive_x=False, x_dt=None, gp_frac=0.0, in_eng="sync",
         gp_self_dma=False, out_dt=None, merged_out=False, routes3=False):
    # bf16 matmul operands + bf16 xj multiplicand + bf16 output (upconverted
    # to f32 on the host): the kernel is HBM-bound and both the input reads
    # and the dominant output write are halved vs f32. Measured max error vs
    # the fp32 reference is ~6e-3 of output scale (harness gate: 2e-2).
    if mm_dt is None:
        mm_dt = mybir.dt.float32r
    nc = _get_nc(mm_dt, psum_cols, psum_bufs, io_bufs, out_bufs, derive_x, x_dt,
                 gp_frac, in_eng, gp_self_dma, out_dt, merged_out, routes3)
    in_maps = _prep_inputs(x, W, derive_x, mm_dt, x_dt)
    res = run_bass_kernel_spmd(nc, in_maps, core_ids=list(range(NCORES)), trace=trace)
    parts = []
    for c in range(NCORES):
        yd = res.results[c]["y"].astype(np.float32)
        if merged_out:
            yd = _unmerge(yd)
        parts.append(yd.reshape(BL, NPAIR, D))
    return np.concatenate(parts, axis=0), res


BEST_CFG = dict(
    mm_dt=mybir.dt.bfloat16,
    x_dt=mybir.dt.bfloat16,
    out_dt=mybir.dt.bfloat16,
    gp_frac=2.1,
    in_eng="scalar",
    merged_out=True,
    psum_cols=1024,
    psum_bufs=2,
    out_bufs=2,
)


def kernel(x, W):
    y, _ = _run(x, W, **BEST_CFG)
    return y



# revision 38
# speedup vs baseline: 1.1690x; 1.1690x over previous
"""Trainium2 Bass kernel for BiLinearInteractionLayer.

Computes, for every field pair p=(i,j), i<j, of F=32 fields:
    y[b, p, :] = (x[b, i, :] @ W[p].T) * x[b, j, :]
x: [4096, 32, 64] f32, W: [496, 64, 64] f32 -> y: [4096, 496, 64] f32.

Sharding: data-parallel over the batch dim across 8 NeuronCores (512
rows each); the weight stack is replicated.

Per-core algorithm (batch tile of 128 rows at a time):
  - Host pre-transposes layouts and downcasts to bf16 (free): the
    contraction dim d lands on SBUF partitions with contiguous DMAs.
    bf16 inputs + bf16 output (upconverted to f32 on the host) halve
    all HBM traffic; measured error ~7e-3 of output scale vs the 2e-2
    harness gate.
  - For each first-field i, the pairs (i, i+1..31) are contiguous both
    in the pair axis and in the transposed weight columns: one
    stationary xT_i [64d, 128b] serves matmuls streaming W^T columns
    (N<=512 per PSUM bank) into [128, <=1024] PSUM groups. Even fields
    use PE rows 0-63, odd fields rows 64-127.
  - PSUM: 2 tags (parity) x 2 banks x bufs=2 fills all 8 banks and
    breaks the matmul(g+1)-waits-mul(g) ping-pong chain that otherwise
    dominates the span.
  - The elementwise xj multiply is split between DVE (tensor_mul
    straight from PSUM, 1.042 ns/col) and gpsimd (ACT first drains
    PSUM->SBUF since the Q7 cores can't read PSUM; ~1.98 ns/col),
    greedily balanced by measured ns/col. An ACT+DVE-2x_1p bf16 route
    exists (routes3) but loses to SBUF port contention in practice.
  - Outputs stage into one SBUF buffer per parity per tile (parity-major
    HBM layout, unshuffled on the host) and drain in ~quarter-parity
    DMA chunks: 124 -> 40 output DMAs, keeping the SP sequencer (~600ns
    per dma_start) off the critical path. Input loads issue from the
    scalar queue so they are never FIFO-blocked behind output stores.
"""

import itertools

import numpy as np

import concourse.bass as bass
import concourse.mybir as mybir
import concourse.tile as _tile
from concourse.bass_utils import run_bass_kernel_spmd
from concourse.tile import TileContext
from concourse.tile_scheduler import N_PROCS
from concourse.vector_clock import ScopedClock, VectorClock

# --------------------------------------------------------------------------
# Tail-drain patch: the staged walrus rejects >1 sync-wait command on a
# TPB_CTRL (Drain) instruction, but the stock Tile tail-drain attaches one
# wait per outstanding sem lane to a single Drain. Replace it with a ladder
# of single-wait SP nops (one per proc lane) followed by a wait-less drain.
# --------------------------------------------------------------------------


def _split_drain_and_barrier(self, tick_clock, wait_clock):
    nc = self.nc
    g = tick_clock.global_clock
    for p in range(N_PROCS):
        tick = g.peek_next(p) - 1
        if tick <= 0:
            continue
        pc = VectorClock()
        pc.require_at_least(p, tick)
        w = nc.sync.nop(nofuse=True)
        wait_clock.add_sem_waits(w.ins, ScopedClock({None: pc}))
    nc.sync.drain()
    nc.all_engine_barrier()
    assert self.sems is not None
    popped = nc._tile_sem_poison_stack.pop()
    assert popped is self._sem_poison
    nc.clear_and_free_semaphores(list(self.sems.allocated().values()))
    nc.all_engine_barrier()


_tile.TileContext._drain_and_barrier = _split_drain_and_barrier

_wsplit_counter = [0]


def _legalize_single_wait(nc):
    """Hoist extra sem waits onto preceding same-engine NoOps.

    This walrus build encodes at most ONE sync-wait command per TPB
    instruction; Tile's sem-assignment pass freely attaches several.
    Splitting extras onto immediately-preceding NoOps on the same engine
    preserves program order (engines issue in order), hence semantics."""
    import bass_rust

    for fn in nc.m.functions:
        for blk in fn.blocks:
            insts = list(blk.instructions)
            if not any(
                ins.sync_info is not None and len(ins.sync_info.on_wait) > 1
                for ins in insts
            ):
                continue
            out = []
            for ins in insts:
                si = ins.sync_info
                waits = list(si.on_wait) if si is not None else []
                if len(waits) > 1:
                    for w in waits[:-1]:
                        _wsplit_counter[0] += 1
                        nop = mybir.InstNoOp(
                            name=f"I-wsplit-{_wsplit_counter[0]}", ins=[], outs=[]
                        )
                        nop.engine = ins.engine
                        nop.sync_info = bass_rust.SyncInfo(
                            on_wait=[w], on_update=[]
                        )
                        out.append(nop)
                    si.on_wait = [waits[-1]]
                out.append(ins)
            blk.instructions = out


# --------------------------------------------------------------------------
# Problem constants (hardcoded per contract: kernel.py is self-contained).
# --------------------------------------------------------------------------
B, F, D = 4096, 32, 64
NCORES = 8
BL = B // NCORES          # 512 batch rows per core
PT = 128                  # batch tile = SBUF partition count
TILES = BL // PT          # 4 tiles per core
NPAIR = F * (F - 1) // 2  # 496
# pair index of (i, i+1) within itertools.combinations(range(F), 2) order
IDX0 = [0] * F
for _i in range(1, F):
    IDX0[_i] = IDX0[_i - 1] + (F - _i)
# per-parity column offset of field i's run inside its wt half
POFF = [0] * F
for _i in range(2, F):
    POFF[_i] = POFF[_i - 2] + (F - 1 - (_i - 2)) * D
WT_COLS = max(POFF[30] + 1 * D, POFF[31])  # even half is the larger: 16384
WT_COLS = max(WT_COLS, 16384)

F32 = mybir.dt.float32

_nc_cache = {}


PAR_COLS = (16384, 15360)   # total columns per parity half (even-i, odd-i)
PAR_BASE = (0, 16384)       # column base of each parity in the merged layout


def _build_bass(mm_dt=F32, psum_cols=1984, psum_bufs=1, io_bufs=2, out_bufs=3,
                derive_x=False, x_dt=None, gp_frac=0.0, in_eng="sync",
                gp_self_dma=False, out_dt=None, merged_out=False,
                routes3=False):
    if x_dt is None:
        x_dt = F32
    if out_dt is None:
        out_dt = F32
    nc = bass.Bass(trn_type="TRN2")
    if derive_x:
        # x (natural layout) is rebuilt on-chip from xt via PE transposes,
        # saving its 4 MB HBM read; 2 PSUM banks go to transpose staging
        psum_cols = 1536
        x_d = None
        id_d = nc.dram_tensor("ident", [PT, PT], mm_dt, kind="ExternalInput")
    else:
        x_d = nc.dram_tensor("x", [BL, F * D], x_dt, kind="ExternalInput")
    xt_d = nc.dram_tensor("xt", [PT, TILES * (F // 2) * PT], mm_dt,
                          kind="ExternalInput")
    wt_d = nc.dram_tensor("wt", [PT, WT_COLS], mm_dt, kind="ExternalInput")
    y_d = nc.dram_tensor("y", [BL, NPAIR * D], out_dt, kind="ExternalOutput")

    CB = (F // 2) * PT  # 2048 xt cols per batch tile

    with TileContext(nc) as tc:
        # Elementwise-multiply routing. Three pipelines (ns/col measured
        # on HW via microbench, + per-instruction fixed cost):
        #   A: DVE mul straight from PSUM           (DVE 1.042)
        #   B: ACT copy PSUM->bf16 SBUF, DVE mul    (ACT 1.056, DVE 0.521:
        #      all-bf16 SBUF operands engage DVE 2x_1p)
        #   C: ACT copy PSUM->bf16 SBUF, gpsimd mul (ACT 1.056, GP ~1.98)
        # Greedy makespan balance across the three engine clocks.
        eng_t = {"dve": 0.0, "act": 0.0, "gp": 0.0}

        def _route(gcols, allow_b=routes3):
            if gp_frac <= 0.0:
                return "A"
            cost = {
                "A": {"dve": gcols * 1.042 + 150},
                "B": {"act": gcols * 1.056 + 150, "dve": gcols * 0.521 + 150},
                "C": {"act": gcols * 1.056 + 150, "gp": gcols * gp_frac + 450},
            }
            if not allow_b:
                del cost["B"]
            best, best_mk = None, None
            for r, c in cost.items():
                mk = max(
                    eng_t[e] + c.get(e, 0.0) for e in eng_t
                )
                if best_mk is None or mk < best_mk:
                    best, best_mk = r, mk
            for e, v in cost[best].items():
                eng_t[e] += v
            return best

        with (
            tc.tile_pool(name="wtp", bufs=1) as wtp,
            tc.tile_pool(name="iop", bufs=io_bufs) as iop,
            tc.tile_pool(name="outp", bufs=out_bufs) as outp,
            tc.tile_pool(name="projp", bufs=2) as projp,
            tc.tile_pool(name="pp", bufs=psum_bufs, space="PSUM") as pp,
        ):
            # input loads go out on their own queue (in_eng) so they are
            # never FIFO-blocked behind output stores on the sync queue
            ieng = getattr(nc, in_eng)
            if derive_x:
                ident = wtp.tile([PT, PT], mm_dt, tag="id")
                ieng.dma_start(out=ident, in_=id_d[:])
            # tile-0 inputs are issued ahead of the weight chunks so the
            # first matmuls aren't gated on the whole weight stream
            # NOTE: this exact emission sequence (xt0, x0, then 2048-col wt
            # chunks) is load-bearing: reordering it or splitting the first
            # wt chunks finer regresses ~24 us (measured twice), plausibly
            # via DMA-engine round-robin alignment of the big output stores.
            x0_s = None
            xt0_s = iop.tile([PT, CB], mm_dt, tag="xt", name="xt_0")
            ieng.dma_start(out=xt0_s, in_=xt_d[:, 0:CB])
            if not derive_x:
                x0_s = iop.tile([PT, F * D], x_dt, tag="x", name="x_0")
                ieng.dma_start(out=x0_s, in_=x_d[0:PT, :])
            wt_s = wtp.tile([PT, WT_COLS], mm_dt)
            # chunked weight load: matmuls for early fields only depend on
            # their own column range (Tile subtile deps), so compute starts
            # after ~1/8 of the weights have landed instead of all 4 MB
            WCH = 2048
            for w0 in range(0, WT_COLS, WCH):
                ieng.dma_start(
                    out=wt_s[:, w0 : w0 + WCH], in_=wt_d[:, w0 : w0 + WCH]
                )
            for t in range(TILES):
                if t == 0:
                    xt_s = xt0_s
                    x_s = x0_s
                    if derive_x:
                        x_s = iop.tile([PT, F * D], F32, tag="x", name="x_0")
                else:
                    x_s = iop.tile(
                        [PT, F * D], x_dt if not derive_x else F32, tag="x",
                        name=f"x_{t}",
                    )
                    if not derive_x:
                        ieng.dma_start(
                            out=x_s, in_=x_d[t * PT : (t + 1) * PT, :]
                        )
                    xt_s = iop.tile([PT, CB], mm_dt, tag="xt", name=f"xt_{t}")
                    ieng.dma_start(
                        out=xt_s, in_=xt_d[:, t * CB : (t + 1) * CB]
                    )
                if derive_x:
                    # x[b, (2m+par)*64+d] = xt[par*64+d, m*128+b]: one PE
                    # transpose per 128-column chunk, ACT copies PSUM->SBUF
                    for m in range(F // 2):
                        tp = pp.tile(
                            [PT, PT], mm_dt, tag="tp", bufs=2,
                            name=f"tp_{t}_{m}",
                        )
                        nc.tensor.transpose(
                            tp, xt_s[:, m * PT : (m + 1) * PT], ident
                        )
                        nc.scalar.copy(
                            out=x_s[:, m * PT : (m + 1) * PT], in_=tp
                        )
                if merged_out:
                    # one SBUF staging buffer per parity for the whole
                    # tile; groups land at their parity-local POFF offset
                    # and only ~2 big DMAs per parity drain it (the DMA
                    # count is decoupled from the PSUM group size)
                    om = {
                        par: outp.tile(
                            [PT, PAR_COLS[par]], out_dt, tag=f"o{par}",
                            name=f"o_{t}_{par}",
                        )
                        for par in (0, 1)
                    }
                    flushed = {0: 0, 1: 0}

                    def _flush(par, upto, t=t, om=om, flushed=flushed):
                        a = flushed[par]
                        if upto <= a:
                            return
                        nc.sync.dma_start(
                            out=y_d[
                                t * PT : (t + 1) * PT,
                                PAR_BASE[par] + a : PAR_BASE[par] + upto,
                            ],
                            in_=om[par][:, a:upto],
                        )
                        flushed[par] = upto
                for m in range(F // 2):
                    # split each live parity's field run into PSUM-group
                    # work items of <= psum_cols columns, then alternate
                    # parities so the two PE row groups interleave
                    per_par = []
                    for par in (0, 1):
                        i = 2 * m + par
                        if i > F - 2:
                            continue
                        ncol = (F - 1 - i) * D
                        groups = []
                        for g0 in range(0, ncol, psum_cols):
                            gcols = min(psum_cols, ncol - g0)
                            groups.append((par, i, g0, gcols))
                        per_par.append(groups)
                    order = [
                        g
                        for pair in itertools.zip_longest(*per_par)
                        for g in pair
                        if g is not None
                    ]
                    for par, i, g0, gcols in order:
                        lhsT = xt_s[par * D : (par + 1) * D,
                                    m * PT : (m + 1) * PT]
                        off = POFF[i] + g0
                        ps = pp.tile(
                            [PT, psum_cols], F32, tag=f"ps{par}",
                            name=f"ps_{t}_{i}_{g0}",
                        )
                        if merged_out:
                            out_s = om[par][:, off : off + gcols]
                        else:
                            ot = outp.tile(
                                [PT, psum_cols], out_dt, tag=f"o{par}",
                                name=f"o_{t}_{i}_{g0}",
                            )
                            out_s = ot[:, :gcols]
                        for k0 in range(0, gcols, 512):
                            kn = min(512, gcols - k0)
                            nc.tensor.matmul(
                                ps[:, k0 : k0 + kn],
                                lhsT,
                                wt_s[par * D : (par + 1) * D,
                                     off + k0 : off + k0 + kn],
                                start=True,
                                stop=True,
                            )
                        c0 = IDX0[i] * D + g0
                        xj = x_s[:, (i + 1) * D + g0 : (i + 1) * D + g0 + gcols]
                        route = _route(gcols)
                        if route == "A":
                            nc.vector.tensor_mul(
                                out=out_s, in0=ps[:, :gcols], in1=xj
                            )
                        else:
                            proj_s = projp.tile(
                                [PT, psum_cols],
                                out_dt if routes3 else F32,
                                tag=f"pr{par}",
                                name=f"pr_{t}_{i}_{g0}",
                            )
                            nc.scalar.copy(
                                out=proj_s[:, :gcols], in_=ps[:, :gcols]
                            )
                            meng = nc.vector if route == "B" else nc.gpsimd
                            meng.tensor_mul(
                                out=out_s, in0=proj_s[:, :gcols], in1=xj
                            )
                        if merged_out:
                            end = off + gcols
                            # drain the staging buffer in ~quarter chunks so
                            # the output stream starts early in each tile;
                            # eighths on the last tile so the final DMA
                            # after the last mul is small (shorter tail)
                            step = PAR_COLS[par] // 8
                            nxt = (flushed[par] // step + 1) * step
                            if end >= nxt and end < PAR_COLS[par]:
                                _flush(par, end)
                        else:
                            nc.sync.dma_start(
                                out=y_d[
                                    t * PT : (t + 1) * PT, c0 : c0 + gcols
                                ],
                                in_=out_s,
                            )
                if merged_out:
                    for par in (0, 1):
                        _flush(par, PAR_COLS[par])
    _legalize_single_wait(nc)
    return nc


def _get_nc(mm_dt, psum_cols, psum_bufs, io_bufs=2, out_bufs=3, derive_x=False,
            x_dt=None, gp_frac=0.0, in_eng="sync", gp_self_dma=False,
            out_dt=None, merged_out=False, routes3=False):
    key = (str(mm_dt), psum_cols, psum_bufs, io_bufs, out_bufs, derive_x,
           str(x_dt), gp_frac, in_eng, gp_self_dma, str(out_dt), merged_out,
           routes3)
    if key not in _nc_cache:
        _nc_cache[key] = _build_bass(
            mm_dt, psum_cols, psum_bufs, io_bufs, out_bufs, derive_x, x_dt,
            gp_frac, in_eng, gp_self_dma, out_dt, merged_out, routes3,
        )
    return _nc_cache[key]


def _np_dt(dt):
    return mybir.dt.np(dt)


def _prep_inputs(x, W, derive_x=False, mm_dt=F32, x_dt=None):
    if x_dt is None:
        x_dt = F32
    mm_np = _np_dt(mm_dt)
    x_np = _np_dt(x_dt)
    x = np.ascontiguousarray(np.asarray(x, dtype=np.float32))
    W = np.ascontiguousarray(np.asarray(W, dtype=np.float32))
    # wt2[par*64+d, POFF[i] + (j-i-1)*64 + o] = W[(i,j), o, d]
    wt2 = np.zeros((PT, WT_COLS), dtype=np.float32)
    for i in range(F - 1):
        par = i % 2
        npair = F - 1 - i
        blk = W[IDX0[i] : IDX0[i] + npair]           # [npair, D, D]
        blk = blk.transpose(2, 0, 1).reshape(D, npair * D)
        wt2[par * D : (par + 1) * D, POFF[i] : POFF[i] + npair * D] = blk
    wt2 = np.ascontiguousarray(wt2.astype(mm_np))
    in_maps = []
    for c in range(NCORES):
        xl = x[c * BL : (c + 1) * BL]                      # [512, 32, 64]
        x_in = np.ascontiguousarray(xl.reshape(BL, F * D).astype(x_np))
        # xt2[par*64+d, t*2048 + m*128 + b] = xl[t*128+b, 2m+par, d]
        xt2 = np.ascontiguousarray(
            xl.reshape(TILES, PT, F // 2, 2, D).transpose(3, 4, 0, 2, 1)
        ).reshape(PT, TILES * (F // 2) * PT).astype(mm_np)
        xt2 = np.ascontiguousarray(xt2)
        m = {"xt": xt2, "wt": wt2}
        if derive_x:
            m["ident"] = np.eye(PT, dtype=mm_np)
        else:
            m["x"] = x_in
        in_maps.append(m)
    return in_maps


def _unmerge(yd):
    """Undo the parity-major device layout -> reference pair order."""
    out = np.empty((BL, NPAIR * D), dtype=np.float32)
    for i in range(F - 1):
        run = (F - 1 - i) * D
        src = PAR_BASE[i % 2] + POFF[i]
        out[:, IDX0[i] * D : IDX0[i] * D + run] = yd[:, src : src + run]
    return out


def _run(x, W, trace=False, mm_dt=None, psum_cols=1984, psum_bufs=1, io_bufs=2,
         out_bufs=3, derive_x=False, x_dt=None, gp_frac=0.0, in_eng="sync",
         gp_self_dma=False, out_dt=None, merged_out=False, routes3=False):
    # bf16 matmul operands + bf16 xj multiplicand + bf16 output (upconverted
    # to f32 on the host): the kernel is HBM-bound and both the input reads
    # and the dominant output write are halved vs f32. Measured max error vs
    # the fp32 reference is ~6e-3 of output scale (harness gate: 2e-2).
    if mm_dt is None:
        mm_dt = mybir.dt.float32r
    nc = _get_nc(mm_dt, psum_cols, psum_bufs, io_bufs, out_bufs, derive_x, x_dt,
                 gp_frac, in_eng, gp_self_dma, out_dt, merged_out, routes3)
    in_maps = _prep_inputs(x, W, derive_x, mm_dt, x_dt)
    res = run_bass_kernel_spmd(nc, in_maps, core_ids=list(range(NCORES)), trace=trace)
    parts = []
    for c in range(NCORES):
        yd = res.results[c]["y"].astype(np.float32)
        if merged_out:
            yd = _unmerge(yd)
        parts.append(yd.reshape(BL, NPAIR, D))
    return np.concatenate(parts, axis=0), res


BEST_CFG = dict(
    mm_dt=mybir.dt.bfloat16,
    x_dt=mybir.dt.bfloat16,
    out_dt=mybir.dt.bfloat16,
    gp_frac=2.1,
    in_eng="scalar",
    merged_out=True,
    psum_cols=1024,
    psum_bufs=2,
    out_bufs=2,
)


def kernel(x, W):
    y, _ = _run(x, W, **BEST_CFG)
    return y

